# revision 9
# baseline (speedup 1.0000x reference)
"""Trainium2 Bass kernel for BasicInteractionNetworkModule.

Data-parallel over batch (B=16) across 8 NeuronCores, 2 batches/core.

Math (per batch b):
  senders   = S^T @ O          [R, 128]   (S = sender_relations [128, R])
  receivers = R_rel^T @ O      [R, 128]
  rel_x = [senders, receivers, info]   [R, 320]
  h = relu-MLP(rel_x): 320 -> 256 -> 256 -> 256 -> 128 (relu after every layer)
  eff_recv = R_rel @ effects   [128, 128]
  obj_x = [O, ext, eff_recv]   [128, 288]
  out = relu-MLP2(obj_x): 288 -> 256 -> 256 -> 128 (no final relu)

Device strategy (v5): feature-major relation MLP with stationary weights;
layer-1 folds the sender/receiver projections via A_s = O @ rw1[:128],
A_r = O @ rw1[128:256] so S/R stream from DRAM as moving operands.  All
small-K matmuls (info part K=64, ext part K=32) are padded to K=128 with
zeros so the PE never switches row groups (row-group transitions measured
~+130 ns on both sides).  L4 computes effects relation-major (stationary =
H3 slices) so the aggregation can contract over relations; its free-dim
bias is applied off the PE by a DVE tensor_tensor add (PSUM + broadcast
bias) followed by an ACT relu, which removed ~13.5 us of K=1 bias-seed
matmul streaming.  The aggregation accumulates all 127 relation slices of
a batch into one PSUM bank held for the whole batch.  Warm-up matmuls at
t=0 bring the PE HAM clock gate to 8/8 before real work lands; the first
chunk's loads are split and the setup tensors ride the sync queue ahead of
the bulk so the pipeline starts ~11 us in.  Object MLP runs in bf16 (fp32
LDWEIGHTS/matmul measured ~3x slower) with its layer-1 o/x partial matmuls
emitted before the pipeline drain.  Evacuations are split across ACT and
DVE; PSUM: 5x 512-wide L1-L3 accumulators, 2x L4, 1x aggregation.
"""

import numpy as np
import ml_dtypes

B, N_OBJ, N_REL = 16, 128, 16256
OBJ_D, REL_D, EFF_D, EXT_D, OUT_D = 128, 64, 128, 32, 128
HID = 256
N_CORES = 8
B_CORE = B // N_CORES  # 2
M_CHUNK = 1024

_CACHE = {}


def _chunks():
    out = []
    base = 0
    while base < N_REL:
        mc = min(M_CHUNK, N_REL - base)
        if mc == 896:
            out.append((base, 512))
            out.append((base + 512, 384))
            base += 896
        else:
            out.append((base, mc))
            base += mc
    return out


def _mtiles(mc):
    out = []
    base = 0
    while base < mc:
        n = min(512, mc - base)
        out.append((base, n))
        base += n
    return out


def build_kernel():
    from concourse import bacc
    import concourse.mybir as mybir
    import concourse.tile as tile
    from concourse.tile_rust import add_dep_helper

    F32 = mybir.dt.float32
    BF16 = mybir.dt.bfloat16
    RELU = mybir.ActivationFunctionType.Relu
    ADD = mybir.AluOpType.add
    MAX = mybir.AluOpType.max

    nc = bacc.Bacc(None)

    # per-core inputs
    S_d = nc.dram_tensor("s_rel", [B_CORE, N_OBJ, N_REL], BF16, kind="ExternalInput")
    R_d = nc.dram_tensor("r_rel", [B_CORE, N_OBJ, N_REL], BF16, kind="ExternalInput")
    IT_d = nc.dram_tensor("info_t", [B_CORE, 128, N_REL], BF16, kind="ExternalInput")
    RT_d = nc.dram_tensor("r_rel_t", [B_CORE, N_REL, N_OBJ], BF16, kind="ExternalInput")
    OT_d = nc.dram_tensor("objs_t", [B_CORE, OBJ_D, N_OBJ], BF16, kind="ExternalInput")
    XT_d = nc.dram_tensor("ext_t", [B_CORE, 128, N_OBJ], BF16, kind="ExternalInput")

    rw1s_d = nc.dram_tensor("rw1s", [128, HID], BF16, kind="ExternalInput")
    rw1r_d = nc.dram_tensor("rw1r", [128, HID], BF16, kind="ExternalInput")
    rw1i_d = nc.dram_tensor("rw1i", [128, HID], BF16, kind="ExternalInput")
    rw2_d = nc.dram_tensor("rw2f", [128, 2, HID], BF16, kind="ExternalInput")
    rw3_d = nc.dram_tensor("rw3f", [128, 2, HID], BF16, kind="ExternalInput")
    rw4_d = nc.dram_tensor("rw4b", [128, 2, EFF_D], BF16, kind="ExternalInput")
    ow1o_d = nc.dram_tensor("ow1o", [128, HID], BF16, kind="ExternalInput")
    ow1x_d = nc.dram_tensor("ow1x", [128, HID], BF16, kind="ExternalInput")
    ow1e_d = nc.dram_tensor("ow1e", [128, HID], BF16, kind="ExternalInput")
    ow2_d = nc.dram_tensor("ow2f", [128, 2, HID], BF16, kind="ExternalInput")
    ow3_d = nc.dram_tensor("ow3f", [128, 2, OUT_D], BF16, kind="ExternalInput")

    rb1_d = nc.dram_tensor("rb1c", [128, 2], F32, kind="ExternalInput")
    rb2_d = nc.dram_tensor("rb2c", [128, 2], F32, kind="ExternalInput")
    rb3_d = nc.dram_tensor("rb3c", [128, 2], F32, kind="ExternalInput")
    ob1_d = nc.dram_tensor("ob1c", [128, 2], F32, kind="ExternalInput")
    ob2_d = nc.dram_tensor("ob2c", [128, 2], F32, kind="ExternalInput")
    ob3_d = nc.dram_tensor("ob3r", [128, OUT_D], F32, kind="ExternalInput")
    rb4bc_d = nc.dram_tensor("rb4bc", [128, 512], BF16, kind="ExternalInput")

    out_d = nc.dram_tensor("out", [B_CORE, N_OBJ, OUT_D], F32, kind="ExternalOutput")

    with tile.TileContext(nc) as tc:
        with (
            tc.tile_pool(name="wts", bufs=1) as wts,
            tc.tile_pool(name="perb", bufs=2) as perb,
            tc.tile_pool(name="cin", bufs=4) as cin,
            tc.tile_pool(name="acts", bufs=3) as acts,
            tc.tile_pool(name="psL", bufs=5, space="PSUM") as psL,
            tc.tile_pool(name="ps4", bufs=2, space="PSUM") as ps4p,
            tc.tile_pool(name="psa", bufs=1, space="PSUM") as psap,
        ):
            # ---- warm-up: PE busy from ~t=2us so the HAM clock gate goes
            # 8/8 before the first real matmul ----
            warm = wts.tile([128, 512], BF16)
            nc.vector.memset(warm, 0.0)
            psW = psap.tile([128, 512], F32, tag="sm")
            for _ in range(24):
                nc.tensor.matmul(psW, warm[:, :128], warm, start=True, stop=True)

            # ---- setup tensors needed for the very first matmuls go on
            # sync ahead of the chunk-0 bulk ----
            rw1s = wts.tile([128, HID], BF16)
            rw1r = wts.tile([128, HID], BF16)
            OTs, XTs = [], []
            for b in range(B_CORE):
                OT = perb.tile([OBJ_D, N_OBJ], BF16, tag=f"OT{b}")
                XT = perb.tile([128, N_OBJ], BF16, tag=f"XT{b}")
                OTs.append(OT); XTs.append(XT)
            nc.sync.dma_start(OTs[0], OT_d[0])
            nc.sync.dma_start(rw1s, rw1s_d[:])
            nc.sync.dma_start(OTs[1], OT_d[1])
            nc.sync.dma_start(rw1r, rw1r_d[:])

            chs = _chunks()
            Scs, Rcs, Ics, RTcs = {}, {}, {}, {}

            def chunk_dmas(b, ci, split=False):
                base, mc = chs[ci]
                S_c = cin.tile([N_OBJ, M_CHUNK], BF16, tag="S_c")
                R_c = cin.tile([N_OBJ, M_CHUNK], BF16, tag="R_c")
                I_c = cin.tile([128, M_CHUNK], BF16, tag="I_c")
                RT_c = cin.tile([128, M_CHUNK // 128, N_OBJ], BF16, tag="RT_c")
                if split:
                    h = mc // 2
                    for lo, hi in ((0, h), (h, mc)):
                        nc.sync.dma_start(S_c[:, lo:hi], S_d[b, :, base + lo:base + hi])
                        nc.sync.dma_start(R_c[:, lo:hi], R_d[b, :, base + lo:base + hi])
                        nc.sync.dma_start(I_c[:, lo:hi], IT_d[b, :, base + lo:base + hi])
                else:
                    nc.sync.dma_start(S_c[:, :mc], S_d[b, :, base:base + mc])
                    nc.sync.dma_start(R_c[:, :mc], R_d[b, :, base:base + mc])
                    nc.sync.dma_start(I_c[:, :mc], IT_d[b, :, base:base + mc])
                ns = mc // 128
                nc.sync.dma_start(
                    RT_c[:, :ns, :],
                    RT_d[b, base:base + mc, :].rearrange("(s p) o -> p s o", p=128),
                )
                Scs[(b, ci)] = S_c; Rcs[(b, ci)] = R_c
                Ics[(b, ci)] = I_c; RTcs[(b, ci)] = RT_c

            chunk_dmas(0, 0, split=True)

            # ---- small setup tensors on the scalar HWDGE queue ----
            rw1i = wts.tile([128, HID], BF16)
            rb1 = wts.tile([128, 2], F32)
            nc.scalar.dma_start(rw1i, rw1i_d[:])
            nc.scalar.dma_start(rb1, rb1_d[:])
            for b in range(B_CORE):
                nc.scalar.dma_start(XTs[b], XT_d[b])

            chunk_dmas(0, 1)

            # ---- bulk weights on gpsimd (SWDGE), most-urgent first ----
            rw2 = wts.tile([128, 2, HID], BF16)
            rw3 = wts.tile([128, 2, HID], BF16)
            rw4 = wts.tile([128, 2, EFF_D], BF16)
            rb4bc = wts.tile([128, 512], BF16)
            ow1o = wts.tile([128, HID], BF16)
            ow1x = wts.tile([128, HID], BF16)
            ow1e = wts.tile([128, HID], BF16)
            ow2 = wts.tile([128, 2, HID], BF16)
            ow3 = wts.tile([128, 2, OUT_D], BF16)
            rb2 = wts.tile([128, 2], F32)
            rb3 = wts.tile([128, 2], F32)
            ob1 = wts.tile([128, 2], F32)
            ob2 = wts.tile([128, 2], F32)
            ob3 = wts.tile([128, OUT_D], F32)
            for t, dsrc in [(rw2, rw2_d), (rb2, rb2_d),
                            (rw3, rw3_d), (rb3, rb3_d), (rw4, rw4_d),
                            (rb4bc, rb4bc_d)]:
                nc.gpsimd.dma_start(t, dsrc[:])
            obj_wt_dmas = [(ow1o, ow1o_d), (ow1x, ow1x_d), (ow1e, ow1e_d),
                           (ow2, ow2_d), (ow3, ow3_d),
                           (ob1, ob1_d), (ob2, ob2_d), (ob3, ob3_d)]

            # ---- A_s / A_r for both batches ----
            Ass, Ars = [], []
            for b in range(B_CORE):
                As = perb.tile([N_OBJ, HID], BF16, tag=f"As{b}")
                Ar = perb.tile([N_OBJ, HID], BF16, tag=f"Ar{b}")
                psA = psap.tile([128, 512], F32, tag="sm")
                nc.tensor.matmul(psA[:, :HID], OTs[b], rw1s, start=True, stop=True)
                nc.vector.tensor_copy(As, psA[:, :HID])
                psA2 = psap.tile([128, 512], F32, tag="sm")
                nc.tensor.matmul(psA2[:, :HID], OTs[b], rw1r, start=True, stop=True)
                nc.vector.tensor_copy(Ar, psA2[:, :HID])
                Ass.append(As); Ars.append(Ar)

            for b in range(B_CORE):
                OT, XT, As, Ar = OTs[b], XTs[b], Ass[b], Ars[b]
                effT = perb.tile([EFF_D, N_OBJ], BF16, tag="effT")
                psagg = psap.tile([128, 512], F32, tag="sm")
                aggfirst = [True]

                def stageA(ci, st):
                    base, mc = chs[ci]
                    if (b, ci) not in Scs:
                        chunk_dmas(b, ci)
                    if b == 0 and ci == 2 and obj_wt_dmas:
                        for t, dsrc in obj_wt_dmas:
                            nc.gpsimd.dma_start(t, dsrc[:])
                        obj_wt_dmas.clear()
                    S_c, R_c = Scs.pop((b, ci)), Rcs.pop((b, ci))
                    I_c = Ics.pop((b, ci))
                    st['RT_c'] = RTcs.pop((b, ci))
                    H1 = acts.tile([128, 2, M_CHUNK], BF16, tag="H1")
                    H2 = acts.tile([128, 2, M_CHUNK], BF16, tag="H2")
                    # L1: out^T halves; padded-K info part opens each bank
                    pls = {}
                    for p2 in range(2):
                        h = slice(p2 * 128, (p2 + 1) * 128)
                        for ti, (mt, n) in enumerate(_mtiles(mc)):
                            sl = slice(mt, mt + n)
                            ps = psL.tile([128, 512], F32, tag="ps")
                            nc.tensor.matmul(ps[:, :n], As[:, h], S_c[:, sl], start=True, stop=False)
                            nc.tensor.matmul(ps[:, :n], Ar[:, h], R_c[:, sl], start=False, stop=False)
                            nc.tensor.matmul(ps[:, :n], rw1i[:, h], I_c[:, sl], start=False, stop=True)
                            nc.scalar.activation(H1[:, p2, sl], ps[:, :n], RELU,
                                                 bias=rb1[:, p2:p2 + 1], scale=1.0)
                    # L2: k-outer so the last H1 evac lands before its k1 use
                    for p2 in range(2):
                        for ti, (mt, n) in enumerate(_mtiles(mc)):
                            pls[(p2, ti)] = psL.tile([128, 512], F32, tag="ps", name=f"ps_{p2}_{ti}")
                    for k in range(2):
                        for p2 in range(2):
                            for ti, (mt, n) in enumerate(_mtiles(mc)):
                                sl = slice(mt, mt + n)
                                nc.tensor.matmul(pls[(p2, ti)][:, :n], rw2[:, k, p2 * 128:(p2 + 1) * 128],
                                                 H1[:, k, sl], start=(k == 0), stop=(k == 1))
                    for p2 in range(2):
                        for ti, (mt, n) in enumerate(_mtiles(mc)):
                            sl = slice(mt, mt + n)
                            ps = pls[(p2, ti)]
                            if ti == 0:
                                nc.scalar.activation(H2[:, p2, sl], ps[:, :n], RELU,
                                                     bias=rb2[:, p2:p2 + 1], scale=1.0)
                            else:
                                nc.vector.tensor_scalar(H2[:, p2, sl], ps[:, :n],
                                                        rb2[:, p2:p2 + 1], 0.0, ADD, MAX)
                    st['H2'] = H2

                def stageB(ci, st):
                    base, mc = chs[ci]
                    H2 = st['H2']
                    H3 = acts.tile([128, 2, M_CHUNK], BF16, tag="H3")
                    E3 = acts.tile([128, M_CHUNK], BF16, tag="E3")
                    ns = mc // 128
                    # L3, k-outer
                    pls = {}
                    for p2 in range(2):
                        for ti, (mt, n) in enumerate(_mtiles(mc)):
                            pls[(p2, ti)] = psL.tile([128, 512], F32, tag="ps", name=f"ps_{p2}_{ti}")
                    for k in range(2):
                        for p2 in range(2):
                            for ti, (mt, n) in enumerate(_mtiles(mc)):
                                sl = slice(mt, mt + n)
                                nc.tensor.matmul(pls[(p2, ti)][:, :n], rw3[:, k, p2 * 128:(p2 + 1) * 128],
                                                 H2[:, k, sl], start=(k == 0), stop=(k == 1))
                    for p2 in range(2):
                        for ti, (mt, n) in enumerate(_mtiles(mc)):
                            sl = slice(mt, mt + n)
                            nc.vector.tensor_scalar(H3[:, p2, sl], pls[(p2, ti)][:, :n],
                                                    rb3[:, p2:p2 + 1], 0.0, ADD, MAX)
                    # L4: relation-major effects; bias seeded by padded-K matmul
                    for g in range(0, ns, 4):
                        ge = min(g + 4, ns)
                        span = (ge - g) * 128
                        gsl = slice(g * 128, g * 128 + span)
                        ps4 = ps4p.tile([128, 512], F32, tag="sm")
                        Etmp = acts.tile([128, 512], BF16, tag="Etmp")
                        for k in range(2):
                            for sj in range(g, ge):
                                sl = slice(sj * 128, (sj + 1) * 128)
                                psl = slice((sj - g) * 128, (sj - g + 1) * 128)
                                nc.tensor.matmul(ps4[:, psl], H3[:, k, sl], rw4[:, k, :],
                                                 start=(k == 0 and sj == g),
                                                 stop=(k == 1 and sj == ge - 1),
                                                 skip_group_check=True)
                        nc.vector.tensor_tensor(Etmp[:, :span], ps4[:, :span],
                                                rb4bc[:, :span], ADD)
                        nc.scalar.activation(E3[:, gsl], Etmp[:, :span], RELU, bias=0.0, scale=1.0)
                    st['E3'] = E3

                def stageC(ci, st):
                    base, mc = chs[ci]
                    ns = mc // 128
                    E3 = st['E3']; RT_c = st['RT_c']
                    last_chunk = (ci == len(chs) - 1)
                    for sj in range(ns):
                        nc.tensor.matmul(psagg[:, :N_OBJ], E3[:, sj * 128:(sj + 1) * 128],
                                         RT_c[:, sj, :],
                                         start=aggfirst[0],
                                         stop=(last_chunk and sj == ns - 1),
                                         skip_group_check=True)
                        aggfirst[0] = False

                # 3-stage software pipeline
                sts = [dict() for _ in chs]
                for ci in range(len(chs)):
                    stageA(ci, sts[ci])
                    if ci >= 1:
                        stageB(ci - 1, sts[ci - 1])
                    if ci >= 3:
                        stageC(ci - 3, sts[ci - 3])
                stageB(len(chs) - 1, sts[-1])
                # object-MLP layer-1 partials (no effT dep) fill the drain
                G1 = perb.tile([128, 2, N_OBJ], BF16, tag="G1")
                G2 = perb.tile([128, 2, N_OBJ], BF16, tag="G2")
                psG = []
                for p2 in range(2):
                    ps = ps4p.tile([128, 512], F32, tag="sm", name=f"psG{p2}")
                    h = slice(p2 * 128, (p2 + 1) * 128)
                    nc.tensor.matmul(ps[:, :N_OBJ], ow1o[:, h], OT, start=True, stop=False,
                                     skip_group_check=True)
                    nc.tensor.matmul(ps[:, :N_OBJ], ow1x[:, h], XT, start=False, stop=False,
                                     skip_group_check=True)
                    psG.append(ps)
                stageC(len(chs) - 3, sts[-3])
                stageC(len(chs) - 2, sts[-2])
                stageC(len(chs) - 1, sts[-1])
                nc.vector.tensor_copy(effT, psagg[:, :N_OBJ])

                # ---- object MLP (bf16) ----
                for p2 in range(2):
                    h = slice(p2 * 128, (p2 + 1) * 128)
                    nc.tensor.matmul(psG[p2][:, :N_OBJ], ow1e[:, h], effT, start=False, stop=True,
                                     skip_group_check=True)
                    nc.scalar.activation(G1[:, p2, :], psG[p2][:, :N_OBJ], RELU,
                                         bias=ob1[:, p2:p2 + 1], scale=1.0)
                for p2 in range(2):
                    ps = ps4p.tile([128, 512], F32, tag="sm", name=f"psG2{p2}")
                    h = slice(p2 * 128, (p2 + 1) * 128)
                    nc.tensor.matmul(ps[:, :N_OBJ], ow2[:, 0, h], G1[:, 0, :], start=True, stop=False)
                    nc.tensor.matmul(ps[:, :N_OBJ], ow2[:, 1, h], G1[:, 1, :], start=False, stop=True)
                    nc.scalar.activation(G2[:, p2, :], ps[:, :N_OBJ], RELU,
                                         bias=ob2[:, p2:p2 + 1], scale=1.0)
                pso = ps4p.tile([128, 512], F32, tag="sm", name="pso")
                nc.tensor.matmul(pso[:, :OUT_D], G2[:, 0, :], ow3[:, 0, :], start=True, stop=False)
                nc.tensor.matmul(pso[:, :OUT_D], G2[:, 1, :], ow3[:, 1, :], start=False, stop=True)
                ob = perb.tile([N_OBJ, OUT_D], F32, tag="ob")
                nc.vector.tensor_tensor(ob, pso[:, :OUT_D], ob3, mybir.AluOpType.add)
                nc.gpsimd.dma_start(out_d[b], ob)

    nc.compile()
    return nc


def _prep_inputs(objects, sender_relations, receiver_relations, relation_info,
                 external_effect_info, rw1, rb1, rw2, rb2, rw3, rb3, rw4, rb4,
                 ow1, ob1, ow2, ob2, ow3, ob3):
    bf16 = ml_dtypes.bfloat16
    f32 = np.float32

    def a(x):
        return np.ascontiguousarray(np.asarray(x, dtype=f32))

    objects = a(objects); sender_relations = a(sender_relations)
    receiver_relations = a(receiver_relations); relation_info = a(relation_info)
    external_effect_info = a(external_effect_info)
    rw1, rb1, rw2, rb2, rw3, rb3, rw4, rb4 = map(a, (rw1, rb1, rw2, rb2, rw3, rb3, rw4, rb4))
    ow1, ob1, ow2, ob2, ow3, ob3 = map(a, (ow1, ob1, ow2, ob2, ow3, ob3))

    # relation info, transposed and K-padded 64 -> 128 with zeros
    info_t = np.zeros((B, 128, N_REL), dtype=bf16)
    info_t[:, :REL_D, :] = relation_info.transpose(0, 2, 1).astype(bf16)
    s_bf = sender_relations.astype(bf16)
    r_bf = receiver_relations.astype(bf16)
    r_rel_t = np.ascontiguousarray(
        receiver_relations.transpose(0, 2, 1)).astype(bf16)
    objs_t = np.ascontiguousarray(objects.transpose(0, 2, 1)).astype(bf16)
    # ext, transposed and K-padded 32 -> 128
    ext_t = np.zeros((B, 128, N_OBJ), dtype=bf16)
    ext_t[:, :EXT_D, :] = external_effect_info.transpose(0, 2, 1).astype(bf16)

    rw1i_pad = np.zeros((128, HID), dtype=bf16)
    rw1i_pad[:REL_D] = rw1[256:320].astype(bf16)
    ow1x_pad = np.zeros((128, HID), dtype=bf16)
    ow1x_pad[:EXT_D] = ow1[128:160].astype(bf16)
    rb4bc = np.ascontiguousarray(
        np.broadcast_to(np.tile(rb4, 4).astype(bf16)[None, :], (128, 512)))

    shared = {
        "rw1s": rw1[0:128].astype(bf16),
        "rw1r": rw1[128:256].astype(bf16),
        "rw1i": rw1i_pad,
        "rw2f": np.ascontiguousarray(rw2.reshape(2, 128, HID).transpose(1, 0, 2)).astype(bf16),
        "rw3f": np.ascontiguousarray(rw3.reshape(2, 128, HID).transpose(1, 0, 2)).astype(bf16),
        "rw4b": np.ascontiguousarray(rw4.reshape(2, 128, EFF_D).transpose(1, 0, 2)).astype(bf16),
        "ow1o": ow1[0:128].astype(bf16),
        "ow1x": ow1x_pad,
        "ow1e": ow1[160:288].astype(bf16),
        "ow2f": np.ascontiguousarray(ow2.reshape(2, 128, HID).transpose(1, 0, 2)).astype(bf16),
        "ow3f": np.ascontiguousarray(ow3.reshape(2, 128, OUT_D).transpose(1, 0, 2)).astype(bf16),
        "rb1c": np.ascontiguousarray(rb1.reshape(2, 128).T),
        "rb2c": np.ascontiguousarray(rb2.reshape(2, 128).T),
        "rb3c": np.ascontiguousarray(rb3.reshape(2, 128).T),
        "ob1c": np.ascontiguousarray(ob1.reshape(2, 128).T),
        "ob2c": np.ascontiguousarray(ob2.reshape(2, 128).T),
        "ob3r": np.ascontiguousarray(np.broadcast_to(ob3[None, :], (128, OUT_D))),
        "rb4bc": rb4bc,
    }

    in_maps = []
    for c in range(N_CORES):
        sl = slice(c * B_CORE, (c + 1) * B_CORE)
        m = dict(shared)
        m["s_rel"] = s_bf[sl]
        m["r_rel"] = r_bf[sl]
        m["info_t"] = np.ascontiguousarray(info_t[sl])
        m["r_rel_t"] = r_rel_t[sl]
        m["objs_t"] = objs_t[sl]
        m["ext_t"] = np.ascontiguousarray(ext_t[sl])
        in_maps.append(m)
    return in_maps


def run(in_maps, **spmd_kwargs):
    from concourse.bass_utils import run_bass_kernel_spmd

    if "nc" not in _CACHE:
        _CACHE["nc"] = build_kernel()
    return run_bass_kernel_spmd(_CACHE["nc"], in_maps,
                                core_ids=list(range(N_CORES)), **spmd_kwargs)


def kernel(**inputs) -> np.ndarray:
    in_maps = _prep_inputs(**inputs)
    res = run(in_maps)
    out = np.concatenate([r["out"].reshape(-1, OUT_D) for r in res.results], axis=0)
    return np.ascontiguousarray(out, dtype=np.float32)
